# revision 54
# baseline (speedup 1.0000x reference)
"""Trainium2 Bass kernel for nn_Attention_39402029974027.

Dual-level Bahdanau attention with coverage + ragged per-sentence
renormalization.  Data-parallel over the batch: 16 rows -> 8 NeuronCores,
2 rows per core, no collectives.  The heavy tensors (encoder_feature and
encoder_outputs, 8 MB + 8 MB per core) stream through SBUF exactly once.

Numerics are fp32-faithful: large PE contractions run as bf16 hi+lo
split pairs (exact to ~2^-17 relative, 1 cycle/row vs fp32's 4), the
tanh argument is assembled in fp32 (PE rank-3 update + DVE add), and
softmax skips max-subtraction (|score| <= ||v||_1 ~ 8, exp cannot
overflow) so exp/Z is algebraically identical to the reference.

Per 128-token chunk (tokens on partitions):
  PE : psum = [cov;1;1]^T @ [w_c; dec_hi; dec_lo]      (rank-3, bf16)
  DVE: x    = F + psum                                  (scalar_tensor_tensor)
  ACT: e    = tanh(x)
  DVE: score= sum(e * v_bcast)          (scalar_tensor_tensor accum_out;
                                         NOTE tensor_tensor_reduce hangs
                                         the device in this environment)
Softmax, segment sums (one-hot S matrices built on host from
enc_sent_pos), context c_t (3-term bf16 hi/lo contraction), the ragged
renormalization gather (S^T one-hot matmuls), and coverage update all run
as small PE/DVE/ACT ops per row, software-pipelined in 4-chunk units so
the E-stream contraction overlaps the score pipeline.  Emission order is
tuned against the instruction-cost timeline model (engines execute their
streams in FIFO order, so compute-dependent DMAs are kept off the input
queues' heads).
"""

import numpy as np
from contextlib import ExitStack

B, T, SENT, N = 16, 2048, 64, 512
NCORES = 8
RPC = B // NCORES          # rows per core
NCH = T // 128             # 16 token chunks per row
HALF = NCH // 2            # chunks per DMA batch

_cache = {}

# packed f32 constant layout (free-dim offsets in "cpk" [128, CPK_W])
OF_VB, OF_SVB, OF_IDF, OF_FS, OF_ES = 0, 512, 1024, 1152, 1664
OF_M2T, OF_ONESC, OF_M2, OF_ONES16, OF_MASKS = 2176, 2178, 2179, 2307, 2323
CPK_W = 2324
# packed bf16 constant layout ("cpkb" [128, CPKB_W])
OB_WCB, OB_M4, OB_BW4, OB_BS4, OB_ID24, OB_STH, OB_STL = \
    0, 512, 640, 1152, 1664, 1666, 1682
CPKB_W = 1698


def _f32(x):
    return np.ascontiguousarray(x, dtype=np.float32)


def _split_bf16(x):
    """x (f32) -> (hi, lo) bf16 with hi + lo == x to ~2^-17 rel."""
    import ml_dtypes
    hi = np.ascontiguousarray(x, dtype=np.float32).astype(ml_dtypes.bfloat16)
    lo = (np.asarray(x, dtype=np.float32) - hi.astype(np.float32)).astype(
        ml_dtypes.bfloat16)
    return hi, lo


def _build():
    if "nc" in _cache:
        return _cache["nc"]
    import os
    lvl = int(os.environ.get("BISECT_LEVEL", "3"))

    import concourse.bass as bass
    import concourse.tile as tile
    from concourse import bacc, mybir

    f32 = mybir.dt.float32
    bf16 = mybir.dt.bfloat16
    AF = mybir.ActivationFunctionType
    OP = mybir.AluOpType

    nc = bacc.Bacc("TRN2", target_bir_lowering=False, debug=False,
                   enable_asserts=False, num_devices=NCORES)

    def inp(name, shape, dt=f32):
        return nc.dram_tensor(name, shape, dt, kind="ExternalInput").ap()

    def outp(name, shape, dt=f32):
        return nc.dram_tensor(name, shape, dt, kind="ExternalOutput").ap()

    # --- inputs (per core) ---
    Ff = inp("Ff", [RPC, T, N])           # encoder_feature (f32)
    Eh = inp("Eh", [RPC, T, N], bf16)     # encoder_outputs hi/lo
    El = inp("El", [RPC, T, N], bf16)
    Sm = inp("Sm", [RPC, T, SENT], bf16)  # one-hot seg matrix
    STm = inp("STm", [RPC, SENT, T], bf16)  # its transpose
    cov = inp("cov", [RPC, T])            # coverage (f32, for coverage_new)
    covb = inp("covb", [RPC, 3, T], bf16)  # rows: [cov, ones, ones] per row
    maskP = inp("maskP", [RPC, 128, NCH])  # word mask, chunk-major permuted
    WdTh = inp("WdTh", [N, N], bf16)      # W_dec^T hi/lo
    WdTl = inp("WdTl", [N, N], bf16)
    WsTh = inp("WsTh", [N, N], bf16)      # W_sent_dec^T hi/lo
    WsTl = inp("WsTl", [N, N], bf16)
    cpk = inp("cpk", [128, CPK_W])        # packed f32 constants
    cpkb = inp("cpkb", [128, CPKB_W], bf16)  # packed bf16 constants
    w6b = inp("w6b", [3, N], bf16)        # rank-3 lhsT base (w_c in row 0)

    # --- outputs (per core) ---
    ct_o = outp("ct", [RPC, N])
    attn_o = outp("attn", [RPC, T])
    covn_o = outp("covn", [RPC, T])
    sct_o = outp("sct", [RPC, N])
    sattn_o = outp("sattn", [1, 128])

    with tile.TileContext(nc) as tc, ExitStack() as ctx:
        con = ctx.enter_context(tc.tile_pool(name="con", bufs=1))
        fpool = ctx.enter_context(tc.tile_pool(name="fpool", bufs=5))
        epool = ctx.enter_context(tc.tile_pool(name="epool", bufs=4))
        spool = ctx.enter_context(tc.tile_pool(name="spool", bufs=2))
        epool2 = ctx.enter_context(tc.tile_pool(name="epool2", bufs=5))
        xpool = ctx.enter_context(tc.tile_pool(name="xpool", bufs=3))
        scr = ctx.enter_context(tc.tile_pool(name="scr", bufs=3))
        rowp = ctx.enter_context(tc.tile_pool(name="rowp", bufs=2))
        psx = ctx.enter_context(tc.tile_pool(name="psx", bufs=2, space="PSUM"))
        psu = ctx.enter_context(tc.tile_pool(name="psu", bufs=2, space="PSUM"))
        psg = ctx.enter_context(tc.tile_pool(name="psg", bufs=2, space="PSUM"))
        psf = ctx.enter_context(tc.tile_pool(name="psf", bufs=2, space="PSUM"))

        def ctile(ap_in, shape, dt, name, eng=None):
            t = con.tile(shape, dt, tag=name)
            (eng or nc.scalar).dma_start(t[:], ap_in)
            return t

        # ---- constants into SBUF: two packed DMAs + slice views ----
        cpkb_sb = con.tile([128, CPKB_W], bf16, tag="cpkb")
        nc.scalar.dma_start(cpkb_sb[:], cpkb[:])
        cpk_sb = con.tile([128, CPK_W], f32, tag="cpk")
        nc.scalar.dma_start(cpk_sb[:], cpk[:])
        wdh = ctile(WdTh.rearrange("(c p) n -> p c n", p=128), [128, 4, N],
                    bf16, "wdh", nc.sync)
        wdl = ctile(WdTl.rearrange("(c p) n -> p c n", p=128), [128, 4, N],
                    bf16, "wdl", nc.sync)
        wsh = ctile(WsTh.rearrange("(c p) n -> p c n", p=128), [128, 4, N], bf16, "wsh")
        wsl = ctile(WsTl.rearrange("(c p) n -> p c n", p=128), [128, 4, N], bf16, "wsl")

        vb_sb = cpk_sb[:, OF_VB:OF_VB + N]
        svb_sb = cpk_sb[:, OF_SVB:OF_SVB + N]
        idf_sb = cpk_sb[:, OF_IDF:OF_IDF + 128]
        fs_sb = cpk_sb[:, OF_FS:OF_FS + N]
        es_sb = cpk_sb[:, OF_ES:OF_ES + N]
        m2t_sb = cpk_sb[:, OF_M2T:OF_M2T + 2]
        onesc_sb = cpk_sb[:, OF_ONESC:OF_ONESC + 1]
        m2_sb = cpk_sb[0:2, OF_M2:OF_M2 + 128]
        ones16_sb = cpk_sb[0:1, OF_ONES16:OF_ONES16 + NCH]
        masks_sb = cpk_sb[:, OF_MASKS:OF_MASKS + 1]
        wcb_sb = cpkb_sb[0:1, OB_WCB:OB_WCB + N]
        m4_sb = cpkb_sb[0:4, OB_M4:OB_M4 + 128]
        bw_sb = cpkb_sb[0:4, OB_BW4:OB_BW4 + N]
        bs_sb = cpkb_sb[0:4, OB_BS4:OB_BS4 + N]
        id24_sb = cpkb_sb[0:4, OB_ID24:OB_ID24 + 2]

        def sth_sl(c, a, b):
            return cpkb_sb[:, OB_STH + 4 * c + a:OB_STH + 4 * c + b]

        def stl_sl(c, a, b):
            return cpkb_sb[:, OB_STL + 4 * c + a:OB_STL + 4 * c + b]

        # ---- word decoder projection: dec_w = s @ W_dec^T + b ----
        # (hi/lo split matmuls; the bias rides the same PSUM group as a
        # K=4 rank-add of [bw_hi; bw_lo]; dec_s is emitted after phase A
        # since only dec_w gates the word pipeline)
        dec_w_ps = psf.tile([2, N], f32, tag="fin")
        for c in range(4):
            nc.tensor.matmul(dec_w_ps[:], sth_sl(c, 0, 2), wdh[:, c, :],
                             start=(c == 0), stop=False)
            nc.tensor.matmul(dec_w_ps[:], stl_sl(c, 0, 2), wdh[:, c, :],
                             start=False, stop=False)
        nc.tensor.matmul(dec_w_ps[:], id24_sb, bw_sb, start=False, stop=False)
        for c in range(4):
            nc.tensor.matmul(dec_w_ps[:], sth_sl(c, 0, 2), wdl[:, c, :],
                             start=False, stop=(c == 3))

        # hi/lo split, column-adjacent so one SWDGE DMA can scatter it
        dwhl = con.tile([2, 2 * N], bf16, tag="dwhl")
        nc.vector.tensor_copy(dwhl[:, 0:N], dec_w_ps[:])
        nc.vector.tensor_sub(dwhl[:, N:2 * N], dec_w_ps[:], dwhl[:, 0:N])

        # per-word-row rank-3 rhs rows [w_c ; dec_hi ; dec_lo] live in one
        # [6, N] tile (w_c preloaded from the host; dec rows DMA'd in)
        wd3, w3_t = [], []
        for r in range(RPC):
            w3 = con.tile([3, N], bf16, tag=f"w3_{r}", name=f"w3_{r}")
            nc.scalar.dma_start(w3[:], w6b[:])
            w3_t.append(w3)
            wd3.append(w3[:])

        def emit_w3_fill():
            # dec hi/lo rows into the rank-3 lhsT tiles; on the sync queue
            # right after the first F chunk so the SP sequencer (idle here)
            # pays the issue cost instead of the ACT sequencer
            for r in range(RPC):
                nc.sync.dma_start(w3_t[r][1:2, :], dwhl[r:r + 1, 0:N])
                nc.sync.dma_start(w3_t[r][2:3, :], dwhl[r:r + 1, N:2 * N])

        # ======== interleaved phase A/B units =============================
        # Unit (r, h): DMA one F half, compute its 8 scores (PE rank-3 ->
        # DVE F-add -> ACT tanh -> DVE fused dot, ttr lagged 2 tiles);
        # then immediately exp/split that half and stream its E chunks into
        # the u / segs accumulations.  Z + c_t close per row; the ragged
        # renorm runs last.  dec_s/sentence are emitted mid-stream so the
        # scheduler cannot place them ahead of the word pipeline.
        scores_l = [rowp.tile([128, NCH], f32, tag="scores",
                              name=f"scores{r}") for r in range(RPC)]
        expm_l = [rowp.tile([128, NCH], f32, tag="expm",
                            name=f"expm{r}") for r in range(RPC)]
        cvb_l, st_acc = [], {}
        mp_l, ssb_l = [], []
        for r in range(RPC):
            cvb = rowp.tile([3, T], bf16, tag="cvb")
            nc.gpsimd.dma_start(cvb[:], covb[r])
            cvb_l.append(cvb)
            mp = rowp.tile([128, NCH], f32, tag="mp", name=f"mp{r}")
            nc.gpsimd.dma_start(mp[:], maskP[r])
            mp_l.append(mp)
            s_sb = spool.tile([128, NCH, SENT], bf16, tag="s", name=f"ssb{r}")
            nc.gpsimd.dma_start(s_sb[:],
                                Sm[r].rearrange("(c p) s -> p c s", p=128))
            ssb_l.append(s_sb)

        def emit_F(r, c0, c1):
            nch = c1 - c0
            fft = fpool.tile([128, nch, N], f32, tag="ff")
            nc.sync.dma_start(
                fft[:], Ff[r].rearrange("(c p) n -> p c n", p=128)
                [:, c0:c1, :])
            return fft

        def emit_A(r, c0, c1, fft=None):
            scores, pend = scores_l[r], []
            nch = c1 - c0
            if fft is None:
                fft = emit_F(r, c0, c1)
            for cc in range(nch):
                c = c0 + cc
                xps = psx.tile([128, N], f32, tag="x")
                nc.tensor.matmul(xps[:], cvb_l[r][:, bass.ts(c, 128)],
                                 wd3[r], start=True, stop=True)
                xsb = xpool.tile([128, N], f32, tag="xsb")
                nc.vector.scalar_tensor_tensor(
                    out=xsb[:], in0=fft[:, cc, :], scalar=0.0,
                    in1=xps[:], op0=OP.bypass, op1=OP.add)
                et = epool2.tile([128, N], f32, tag="e")
                nc.scalar.activation(et[:], xsb[:], AF.Tanh)
                pend.append((et, c))
                if len(pend) > 2:
                    pet, pc = pend.pop(0)
                    pscr = scr.tile([128, N], f32, tag="pscr")
                    nc.vector.scalar_tensor_tensor(
                        out=pscr[:], in0=pet[:], scalar=0.0, in1=vb_sb,
                        op0=OP.bypass, op1=OP.mult,
                        accum_out=scores[:, pc:pc + 1])
            for pet, pc in pend:
                pscr = scr.tile([128, N], f32, tag="pscr")
                nc.vector.scalar_tensor_tensor(
                    out=pscr[:], in0=pet[:], scalar=0.0, in1=vb_sb,
                    op0=OP.bypass, op1=OP.mult,
                    accum_out=scores[:, pc:pc + 1])

        def emit_Bh(r, c0, c1):
            """exp/mask/split chunk range; stream its E chunks into u, segs."""
            scores, expm = scores_l[r], expm_l[r]
            nch = c1 - c0
            exps = rowp.tile([128, nch], f32, tag="exps")
            nc.scalar.activation(exps[:], scores[:, c0:c1], AF.Exp)
            nc.vector.tensor_mul(expm[:, c0:c1], exps[:],
                                 mp_l[r][:, c0:c1])
            emh = rowp.tile([128, nch], bf16, tag="emh")
            nc.vector.tensor_copy(emh[:], expm[:, c0:c1])
            eml = rowp.tile([128, nch], bf16, tag="eml")
            nc.vector.tensor_sub(eml[:], expm[:, c0:c1], emh[:])

            s_sb = ssb_l[r]
            if c0 == 0:
                st_acc[r] = (psg.tile([SENT, 1], f32, tag="segs",
                                      name=f"segs{r}"),
                             psu.tile([1, N], f32, tag="u", name=f"ups{r}"))
            segs, ups = st_acc[r]
            eht = epool.tile([128, nch, N], bf16, tag="eh")
            nc.sync.dma_start(
                eht[:], Eh[r].rearrange("(c p) n -> p c n", p=128)
                [:, c0:c1, :])
            elt = epool.tile([128, nch, N], bf16, tag="el")
            nc.sync.dma_start(
                elt[:], El[r].rearrange("(c p) n -> p c n", p=128)
                [:, c0:c1, :])
            for cc in range(nch):
                first = c0 == 0 and cc == 0
                last = c1 == NCH and cc == nch - 1
                nc.tensor.matmul(segs[:], s_sb[:, c0 + cc, :],
                                 emh[:, cc:cc + 1], start=first, stop=last)
                nc.tensor.matmul(ups[:], emh[:, cc:cc + 1], eht[:, cc, :],
                                 start=first, stop=False)
                nc.tensor.matmul(ups[:], emh[:, cc:cc + 1], elt[:, cc, :],
                                 start=False, stop=False)
                nc.tensor.matmul(ups[:], eml[:, cc:cc + 1], eht[:, cc, :],
                                 start=False, stop=last)
            return emh

        def emit_Z(r):
            """softmax normalizer + c_t scale for row r."""
            expm = expm_l[r]
            zp = psf.tile([1, NCH], f32, tag="fin")
            nc.tensor.matmul(zp[:], onesc_sb, expm[:], start=True, stop=True)
            z1 = rowp.tile([1, 1], f32, tag="z1")
            nc.vector.tensor_reduce(z1[:], zp[:], axis=mybir.AxisListType.X,
                                    op=OP.add)
            invz = rowp.tile([1, 1], f32, tag="invz")
            nc.vector.reciprocal(invz[:], z1[:])
            bc16 = psf.tile([NCH, 1], f32, tag="fin")
            nc.tensor.matmul(bc16[:], ones16_sb, invz[:], start=True,
                             stop=True)
            invz16 = rowp.tile([NCH, 1], f32, tag="invz16")
            nc.scalar.copy(invz16[:], bc16[:])
            _, ups = st_acc[r]
            ct_sb = rowp.tile([1, N], f32, tag="ct_sb")
            nc.scalar.mul(ct_sb[:], ups[:], invz[0:1, 0:1])
            nc.gpsimd.dma_start(ct_o[r].unsqueeze(0), ct_sb[:])
            return invz16

        sts_l, covt_l = {}, {}

        def emit_tail_loads(r):
            sts = spool.tile([SENT, T], bf16, tag="st2", name=f"sts{r}")
            nc.gpsimd.dma_start(sts[:], STm[r])
            sts_l[r] = sts
            covt = rowp.tile([NCH, 128], f32, tag="covt", name=f"covt{r}")
            nc.gpsimd.dma_start(covt[:],
                                cov[r].rearrange("(c p) -> c p", p=128))
            covt_l[r] = covt

        def emit_B3_pair(iz16):
            """ragged renorm + outputs, both rows stage-interleaved."""
            ratb_l, sw_l, est_l, swt_l = {}, {}, {}, {}
            segs_sb_l, rec_l = {}, {}
            for r in range(RPC):
                segs, _ = st_acc[r]
                segs_sb = rowp.tile([SENT, 1], f32, tag="segs_sb",
                                    name=f"segsb{r}")
                nc.vector.tensor_scalar_max(segs_sb[:], segs[:], 1e-30)
                segs_sb_l[r] = segs_sb
            for r in range(RPC):
                rec = rowp.tile([SENT, 1], f32, tag="rec", name=f"rec{r}")
                nc.vector.reciprocal(rec[:], segs_sb_l[r][:])
                rec_l[r] = rec
            sar1 = rowp.tile([SENT, 1], f32, tag="sar")
            nc.scalar.dma_start(sar1[:], sattn_sb[SENT:2 * SENT, :])
            for r in range(RPC):
                sar = sattn_sb[0:SENT, :] if r == 0 else sar1[:]
                ratb = rowp.tile([SENT, 1], bf16, tag="ratb",
                                 name=f"ratb{r}")
                nc.vector.tensor_mul(ratb[:], rec_l[r][:], sar)
                ratb_l[r] = ratb
            g2_l = {}
            for r in range(RPC):
                g2 = psf.tile([128, NCH], f32, tag="fin", name=f"g2_{r}")
                for c in range(NCH):
                    nc.tensor.matmul(g2[:, c:c + 1],
                                     sts_l[r][:, bass.ts(c, 128)],
                                     ratb_l[r][:], start=True, stop=True)
                g2_l[r] = g2
                sw = rowp.tile([128, NCH], f32, tag="sw", name=f"sw{r}")
                nc.vector.tensor_mul(sw[:], expm_l[r][:], g2[:])
                sw_l[r] = sw
            for r in range(RPC):
                est = psf.tile([NCH, 128], f32, tag="fin", name=f"est{r}")
                nc.tensor.transpose(est[:], expm_l[r][:], idf_sb)
                est_l[r] = est
                attn_t = rowp.tile([NCH, 128], f32, tag="attn_t",
                                   name=f"attnt{r}")
                nc.scalar.mul(attn_t[:], est[:], iz16[r][:, 0:1])
                nc.sync.dma_start(
                    attn_o[r].rearrange("(c p) -> c p", p=128), attn_t[:])
            for r in range(RPC):
                swt = psf.tile([NCH, 128], f32, tag="fin", name=f"swt{r}")
                nc.tensor.transpose(swt[:], sw_l[r][:], idf_sb)
                covnt = rowp.tile([NCH, 128], f32, tag="covnt",
                                  name=f"covnt{r}")
                nc.vector.tensor_add(covnt[:], covt_l[r][:], swt[:])
                nc.sync.dma_start(
                    covn_o[r].rearrange("(c p) -> c p", p=128), covnt[:])

        def emit_sentence():
            nonlocal sattn_sb
            # sentence decoder projection
            dec_s_ps = psf.tile([2, N], f32, tag="fin")
            for c in range(4):
                nc.tensor.matmul(dec_s_ps[:], sth_sl(c, 2, 4), wsh[:, c, :],
                                 start=(c == 0), stop=False)
                nc.tensor.matmul(dec_s_ps[:], sth_sl(c, 2, 4), wsl[:, c, :],
                                 start=False, stop=False)
                nc.tensor.matmul(dec_s_ps[:], stl_sl(c, 2, 4), wsh[:, c, :],
                                 start=False, stop=False)
            nc.tensor.matmul(dec_s_ps[:], id24_sb, bs_sb,
                             start=False, stop=True)
            dshl = con.tile([2, 2 * N], bf16, tag="dshl")
            nc.vector.tensor_copy(dshl[:, 0:N], dec_s_ps[:])
            nc.vector.tensor_sub(dshl[:, N:2 * N], dec_s_ps[:], dshl[:, 0:N])
            ds4 = con.tile([4, N], bf16, tag="ds4")
            nc.scalar.dma_start(ds4[0:2, :], dshl[:, 0:N])
            nc.scalar.dma_start(ds4[2:4, :], dshl[:, N:2 * N])

            xs = psx.tile([128, N], f32, tag="x")
            nc.tensor.matmul(xs[:], idf_sb, fs_sb, start=True, stop=False)
            nc.tensor.matmul(xs[:], m4_sb, ds4[:], start=False, stop=True)
            es_t = epool2.tile([128, N], f32, tag="e")
            nc.scalar.activation(es_t[:], xs[:], AF.Tanh)
            pscr = scr.tile([128, N], f32, tag="pscr")
            sscore = con.tile([128, 1], f32, tag="sscore")
            nc.vector.scalar_tensor_tensor(
                out=pscr[:], in0=es_t[:], scalar=0.0, in1=svb_sb,
                op0=OP.bypass, op1=OP.mult, accum_out=sscore[:])
            sexp = con.tile([128, 1], f32, tag="sexp")
            nc.scalar.activation(sexp[:], sscore[:], AF.Exp)
            sexpm = con.tile([128, 1], f32, tag="sexpm")
            nc.vector.tensor_mul(sexpm[:], sexp[:], masks_sb)
            zs = psf.tile([2, 1], f32, tag="fin")
            nc.tensor.matmul(zs[:], m2t_sb, sexpm[:], start=True, stop=True)
            invzs = con.tile([2, 1], f32, tag="invzs")
            nc.vector.reciprocal(invzs[:], zs[:])
            bcz = psf.tile([128, 1], f32, tag="fin")
            nc.tensor.matmul(bcz[:], m2_sb, invzs[:], start=True, stop=True)
            sattn_sb = con.tile([128, 1], f32, tag="sattn_sb")
            nc.vector.tensor_mul(sattn_sb[:], sexpm[:], bcz[:])
            satt = psf.tile([1, 128], f32, tag="fin")
            nc.tensor.transpose(satt[:], sattn_sb[:], idf_sb)
            satt_sb = con.tile([1, 128], f32, tag="satt_sb")
            nc.scalar.copy(satt_sb[:], satt[:])
            nc.gpsimd.dma_start(sattn_o[:], satt_sb[:])
            lt = con.tile([128, 2], f32, tag="lt")
            nc.vector.tensor_scalar_mul(lt[:], m2t_sb, sexpm[:, 0:1])
            us = psf.tile([2, N], f32, tag="fin")
            nc.tensor.matmul(us[:], lt[:], es_sb, start=True, stop=True)
            sct_sb = con.tile([2, N], f32, tag="sct_sb")
            nc.scalar.mul(sct_sb[:], us[:], invzs[:, 0:1])
            nc.gpsimd.dma_start(sct_o[:], sct_sb[:])

        sattn_sb = None
        QU = 4
        fft0 = emit_F(0, 0, QU)
        emit_w3_fill()
        for u in range(NCH // QU):
            emit_A(0, QU * u, QU * (u + 1), fft=fft0 if u == 0 else None)
            if u == 0 and lvl >= 1:
                emit_tail_loads(0)
                emit_tail_loads(1)
            if lvl >= 1:
                emit_Bh(0, QU * u, QU * (u + 1))
        if lvl == 0:
            # debug: dump scores as the attn output
            for r in range(RPC):
                nc.sync.dma_start(
                    attn_o[r].rearrange("(a b) -> a b", a=128), scores_l[r][:])
            for r in range(RPC):
                emit_A(1, QU * 0, QU * 1)
                break
        iz0 = emit_Z(0) if lvl >= 1 else None
        if lvl >= 1:
            for u in range(NCH // QU):
                emit_A(1, QU * u, QU * (u + 1))
                emit_Bh(1, QU * u, QU * (u + 1))
        if lvl >= 2:
            emit_sentence()
        iz1 = emit_Z(1) if lvl >= 1 else None
        if lvl >= 3:
            emit_B3_pair({0: iz0, 1: iz1})

    nc.compile()
    _cache["nc"] = nc
    return nc


def _prep_in_maps(inputs):
    import ml_dtypes
    bf = ml_dtypes.bfloat16

    F = _f32(np.asarray(inputs["encoder_feature"]).reshape(B, T, N))
    E = np.asarray(inputs["encoder_outputs"], np.float32).reshape(B, T, N)
    Ehh, Ell = _split_bf16(E)

    pos = np.asarray(inputs["enc_sent_pos"])
    seg = (pos[:, None, :] <= np.arange(T)[None, :, None]).sum(2)  # [B,T]
    idx = np.where(seg < SENT, seg, SENT).astype(np.int64)
    eye = np.eye(SENT + 1, dtype=np.float32)[:, :SENT]
    S = eye[idx]                                  # [B, T, SENT] one-hot
    Sb = S.astype(bf)
    STb = np.ascontiguousarray(S.transpose(0, 2, 1)).astype(bf)

    cov = np.asarray(inputs["coverage"], np.float32)
    covb = np.ones((B, 3, T), np.float32)
    covb[:, 0, :] = cov
    covb = covb.astype(bf)

    mask = np.asarray(inputs["enc_padding_mask"], np.float32)
    maskP = np.ascontiguousarray(
        mask.reshape(B, NCH, 128).transpose(0, 2, 1))

    s_t = np.asarray(inputs["s_t_hat"], np.float32)
    ss_t = np.asarray(inputs["sent_s_t_hat"], np.float32)

    Wd = np.asarray(inputs["W_dec"], np.float32)
    Ws = np.asarray(inputs["W_sent_dec"], np.float32)
    WdTh, WdTl = _split_bf16(Wd.T)
    WsTh, WsTl = _split_bf16(Ws.T)

    # ---- packed f32 constants ----
    cpk = np.zeros((128, CPK_W), np.float32)
    cpk[:, OF_VB:OF_VB + N] = np.asarray(inputs["v_w"], np.float32)[None, :]
    cpk[:, OF_SVB:OF_SVB + N] = np.asarray(inputs["sent_v_w"],
                                           np.float32)[None, :]
    cpk[:, OF_IDF:OF_IDF + 128] = np.eye(128, dtype=np.float32)
    m2 = np.zeros((2, 128), np.float32)
    m2[0, :SENT] = 1.0
    m2[1, SENT:] = 1.0
    cpk[:, OF_M2T:OF_M2T + 2] = m2.T
    cpk[:, OF_ONESC:OF_ONESC + 1] = 1.0
    cpk[0:2, OF_M2:OF_M2 + 128] = m2
    cpk[0:1, OF_ONES16:OF_ONES16 + NCH] = 1.0

    # ---- packed bf16 constants ----
    cpkb = np.zeros((128, CPKB_W), np.float32)
    cpkb[0:1, OB_WCB:OB_WCB + N] = np.asarray(inputs["w_c"],
                                              np.float32)[None, :]
    cpkb[0:4, OB_M4:OB_M4 + 128] = np.concatenate([m2, m2], 0)
    bwh, bwl = _split_bf16(
        np.tile(np.asarray(inputs["b_dec"], np.float32)[None, :], (2, 1)))
    cpkb[0:4, OB_BW4:OB_BW4 + N] = np.concatenate(
        [bwh.astype(np.float32), bwl.astype(np.float32)], 0)
    bsh, bsl = _split_bf16(
        np.tile(np.asarray(inputs["b_sent_dec"], np.float32)[None, :], (2, 1)))
    cpkb[0:4, OB_BS4:OB_BS4 + N] = np.concatenate(
        [bsh.astype(np.float32), bsl.astype(np.float32)], 0)
    cpkb[0:4, OB_ID24:OB_ID24 + 2] = np.concatenate(
        [np.eye(2, dtype=np.float32)] * 2, 0)
    w6b = np.zeros((3, N), np.float32)
    w6b[0] = np.asarray(inputs["w_c"], np.float32)
    w6b = w6b.astype(bf)

    Fs_all = np.asarray(inputs["sent_enc_feature"], np.float32).reshape(
        B, SENT, N)
    Es_all = np.asarray(inputs["sent_enc_outputs"], np.float32)
    maskS_all = np.asarray(inputs["sent_enc_padding_mask"], np.float32)

    in_maps = []
    for i in range(NCORES):
        sl = slice(i * RPC, (i + 1) * RPC)
        cpk_i = cpk.copy()
        cpk_i[:, OF_FS:OF_FS + N] = Fs_all[sl].reshape(128, N)
        cpk_i[:, OF_ES:OF_ES + N] = Es_all[sl].reshape(128, N)
        cpk_i[:, OF_MASKS:OF_MASKS + 1] = maskS_all[sl].reshape(128, 1)
        cpkb_i = cpkb.copy()
        sT = np.stack([s_t[i * RPC], s_t[i * RPC + 1],
                       ss_t[i * RPC], ss_t[i * RPC + 1]], axis=1)  # [N,4]
        sTh, sTl = _split_bf16(sT)
        # chunk-major [128, 4, 4] -> [128, 16]
        cpkb_i[:, OB_STH:OB_STH + 16] = sTh.astype(np.float32).reshape(
            4, 128, 4).transpose(1, 0, 2).reshape(128, 16)
        cpkb_i[:, OB_STL:OB_STL + 16] = sTl.astype(np.float32).reshape(
            4, 128, 4).transpose(1, 0, 2).reshape(128, 16)
        in_maps.append({
            "Ff": np.ascontiguousarray(F[sl]),
            "Eh": np.ascontiguousarray(Ehh[sl]),
            "El": np.ascontiguousarray(Ell[sl]),
            "Sm": np.ascontiguousarray(Sb[sl]),
            "STm": np.ascontiguousarray(STb[sl]),
            "cov": _f32(cov[sl]),
            "covb": np.ascontiguousarray(covb[sl]),
            "maskP": _f32(maskP[sl]),
            "WdTh": WdTh, "WdTl": WdTl, "WsTh": WsTh, "WsTl": WsTl,
            "cpk": cpk_i,
            "cpkb": cpkb_i.astype(bf),
            "w6b": w6b,
        })
    return in_maps


def kernel(**inputs):
    from concourse.bass_utils import run_bass_kernel_spmd

    nc = _build()
    in_maps = _prep_in_maps(inputs)
    res = run_bass_kernel_spmd(nc, in_maps, core_ids=list(range(NCORES)))
    outs = res.results

    c_t = np.concatenate([o["ct"] for o in outs], 0)
    attn = np.concatenate([o["attn"] for o in outs], 0)
    covn = np.concatenate([o["covn"] for o in outs], 0)
    sct = np.concatenate([o["sct"] for o in outs], 0)
    sattn = np.concatenate([o["sattn"].reshape(RPC, SENT) for o in outs], 0)
    return (c_t, attn, covn, sct, sattn)


if __name__ == "__main__":
    import reference
    ins = {k: np.asarray(v) for k, v in reference.setup_inputs().items()}
    outs = kernel(**ins)
    for o in outs:
        print(o.shape, o.dtype)
